# revision 22
# baseline (speedup 1.0000x reference)
"""Trainium2 Bass kernel for the gvlad pooling decoder (nn_Decoder_38182259261791).

Sharding: data-parallel over batch B=32 across 8 NeuronCores (4 images/core).
Per core: conv(7x1)+relu and cluster-score conv are fused into one matmul over
the contracted (C_in*kh)=3584 axis, emitted transposed ([w, c] layout) so the
VLAD aggregation needs no on-chip transpose. BatchNorm batch stats come from an
8-core AllGather of the per-core fc outputs h[4,512]; every core then computes
the full tail (BN + relu + logit) redundantly and core 0's output is returned.
"""

import os

import numpy as np
import ml_dtypes

import concourse.bacc as bacc
import concourse.mybir as mybir
from concourse.tile import TileContext
from concourse.bass_utils import run_bass_kernel_spmd

N_CORES = 8
B_LOC = 4            # images per core
C = 512
H = 7
W = 500
KG = 10              # clusters incl. ghost
K_CL = 8             # kept clusters
NKT = 28             # contraction K-tiles: 4 ci-chunks x 7 kh
MID = 512
OUT = 5994
WT = 4               # w-tiles per image
WSZ = 125            # w-tile size
EPS_BN = 1e-5

BF16 = mybir.dt.bfloat16
F32 = mybir.dt.float32
AF = mybir.ActivationFunctionType
ALU = mybir.AluOpType
BF16_NP = ml_dtypes.bfloat16

_CACHE = {}


def _build(sim=False, with_bias=True):
    nc = bacc.Bacc(
        "TRN2", target_bir_lowering=False, debug=False,
        num_devices=1 if sim else N_CORES,
    )

    x_d = nc.dram_tensor("x", [B_LOC, C, H, W], BF16, kind="ExternalInput")
    w_d = nc.dram_tensor("wt2", [128, NKT * 522], BF16, kind="ExternalInput")
    fcw_d = nc.dram_tensor("fcw2", [128, 32 * 512], BF16, kind="ExternalInput")
    # n-major logit weights: chunk n at [n*2048, (n+1)*2048), kt-major inside
    lw_d = nc.dram_tensor("lw3", [128, 12 * 2048], BF16, kind="ExternalInput")
    # packed f32 params: [0:128) identity, [128:132) gamma, [132:136) beta,
    # [136:200) ones (bf16 x128 via bitcast), [200:461) conv bias row (bf16
    # x522 via bitcast on partition 0), [461:973) centroids (rows 0:10)
    par_d = nc.dram_tensor("par", [128, 973], F32, kind="ExternalInput")

    emb_d = nc.dram_tensor("embT", [MID, 32], F32, kind="ExternalOutput")
    vlad_d = nc.dram_tensor("vlad", [32, OUT], F32, kind="ExternalOutput")

    with TileContext(nc) as tc:
        with (
            tc.tile_pool(name="wp", bufs=1) as wp,
            tc.tile_pool(name="dram", bufs=1, space="DRAM") as dp,
        ):
            # ---- persistent SBUF ----
            w_sb = wp.tile([128, NKT * 522], BF16, tag="w")
            csz = 7 * 522

            def load_w_chunk(i):
                nc.sync.dma_start(
                    out=w_sb[:, i * csz:(i + 1) * csz],
                    in_=w_d.ap()[:, i * csz:(i + 1) * csz],
                )
            par = wp.tile([128, 973], F32, tag="par")

            def load_par():
                nc.sync.dma_start(out=par[:], in_=par_d.ap())

            idf_sb = par[:, 0:128]
            gam_sb = par[:, 128:132]
            bet_sb = par[:, 132:136]
            ones_sb = par[:, 136:200].bitcast(BF16)
            cb_sb = par[0:1, 200:461].bitcast(BF16)
            cent_sb = par[0:KG, 461:973]
            fcw_sb = wp.tile([128, 32 * 512], BF16, tag="fcw")
            lwpre = wp.tile([128, 7 * 2048], BF16, tag="lwpre")
            # embpack: [0:128) emb0T hi, [128:256) emb0T lo, [256:384) embT
            embpack = wp.tile([128, 384], BF16, tag="embpack")
            emb0T_hi = embpack[:, 0:128]
            emb0T_lo = embpack[:, 128:256]
            embT_bf = embpack[:, 256:384]
            cl2all = wp.tile([K_CL, B_LOC * MID], F32, tag="cl2all")

            def load_fc_weights():
                fsz = 8 * 512
                for i in range(4):
                    nc.sync.dma_start(
                        out=fcw_sb[:, i * fsz:(i + 1) * fsz],
                        in_=fcw_d.ap()[:, i * fsz:(i + 1) * fsz],
                    )

            def load_lw_prefix():
                for i in range(2):
                    sz = 7168
                    lo = i * sz // 2 * 2
                    nc.sync.dma_start(
                        out=lwpre[:, i * 7168:(i + 1) * 7168],
                        in_=lw_d.ap()[:, i * 7168:(i + 1) * 7168],
                    )

            # ---- conv + VLAD phase ----
            with (
                tc.tile_pool(name="xp", bufs=8) as xp,
                tc.tile_pool(name="scr", bufs=3) as scr,
                tc.tile_pool(name="ip", bufs=1) as ipp,
                tc.tile_pool(name="pa2", bufs=2, space="PSUM") as pa2,
                tc.tile_pool(name="pa1", bufs=1, space="PSUM") as pa1,
            ):

                def conv_mms(wt, fps, sps, xts):
                    for kt in range(NKT):
                        cc, kh = divmod(kt, 7)
                        lhs = xts[cc][:, kh * W + wt * WSZ: kh * W + (wt + 1) * WSZ]
                        nc.tensor.matmul(
                            fps[:], lhs, w_sb[:, kt * 522: kt * 522 + 512],
                            start=(kt == 0),
                            stop=(not with_bias and kt == NKT - 1),
                            skip_group_check=True,
                        )
                        nc.tensor.matmul(
                            sps[:], lhs, w_sb[:, kt * 522 + 512: (kt + 1) * 522],
                            start=(kt == 0),
                            stop=(not with_bias and kt == NKT - 1),
                            skip_group_check=True,
                        )
                    if with_bias:
                        nc.tensor.matmul(
                            fps[:], ones_sb[0:1, 0:WSZ], cb_sb[0:1, 0:512],
                            start=False, stop=True, skip_group_check=True,
                        )
                        nc.tensor.matmul(
                            sps[:], ones_sb[0:1, 0:WSZ], cb_sb[0:1, 512:522],
                            start=False, stop=True, skip_group_check=True,
                        )

                def postproc(fps, sps):
                    # packed scratch: f bf16 [0:256)f32, e [256:266), rs 266,
                    # inv 267, sa bf16 [268:273)
                    pk = scr.tile([WSZ, 276], F32, tag="pk")
                    f_sb = pk[:, 0:256].bitcast(BF16)
                    nc.scalar.activation(f_sb, fps[:], AF.Relu)
                    e_sb = pk[:, 256:266]
                    rs = pk[:, 266:267]
                    nc.scalar.activation(e_sb, sps[:], AF.Exp, accum_out=rs)
                    inv = pk[:, 267:268]
                    nc.vector.reciprocal(inv, rs)
                    sa_sb = pk[:, 268:273].bitcast(BF16)
                    nc.vector.tensor_scalar_mul(sa_sb, e_sb, inv)
                    return f_sb, sa_sb

                def vlad_mms(wt, f_sb, sa_sb, agg, ssum):
                    nc.tensor.matmul(
                        agg[:], sa_sb, f_sb,
                        start=(wt == 0), stop=(wt == 3), skip_group_check=True,
                    )
                    nc.tensor.matmul(
                        ssum[:], sa_sb, ones_sb[0:WSZ, 0:1],
                        start=(wt == 0), stop=(wt == 3), skip_group_check=True,
                    )

                def image_post(b, agg, ssum):
                    # ip pack (f32): tmp [0:512), res [512:1024), sq [1024:1536),
                    # ss 1536, qs 1537, nrm 1538, nrm2 1539, invn 1540
                    ip = ipp.tile([KG, 1544], F32, tag="ip")
                    ss_sb = ip[:, 1536:1537]
                    nc.vector.tensor_copy(ss_sb, ssum[:])
                    tmp = ip[:, 0:512]
                    nc.vector.tensor_scalar_mul(tmp, cent_sb[:], ss_sb)
                    res = ip[:, 512:1024]
                    nc.vector.tensor_tensor(res, agg[:], tmp, op=ALU.subtract)
                    sq = ip[0:K_CL, 1024:1536]
                    qs = ip[0:K_CL, 1537:1538]
                    nc.scalar.activation(
                        sq, res[0:K_CL, :], AF.Square, accum_out=qs
                    )
                    nrm = ip[0:K_CL, 1538:1539]
                    nc.scalar.activation(nrm, qs, AF.Sqrt)
                    nrm2 = ip[0:K_CL, 1539:1540]
                    nc.vector.tensor_scalar_max(nrm2, nrm, 1e-12)
                    invn = ip[0:K_CL, 1540:1541]
                    nc.vector.reciprocal(invn, nrm2)
                    nc.vector.tensor_scalar_mul(
                        cl2all[:, b * MID:(b + 1) * MID], res[0:K_CL, :], invn
                    )

                def transposes(b):
                    # cl2[b] [8, 512] f32 -> emb0T hi/lo bf16 cols k*16 + cc*4 + b
                    for cc4 in range(4):
                        tp = pa2.tile([128, K_CL], F32, tag="tp", name=f"tp{b}_{cc4}")
                        nc.tensor.transpose(
                            tp[:],
                            cl2all[:, b * MID + cc4 * 128: b * MID + (cc4 + 1) * 128],
                            idf_sb[0:K_CL, 0:K_CL],
                        )
                        dhi = emb0T_hi.rearrange("p (k s) -> p k s", s=16)[
                            :, :, cc4 * 4 + b
                        ]
                        dlo = emb0T_lo.rearrange("p (k s) -> p k s", s=16)[
                            :, :, cc4 * 4 + b
                        ]
                        nc.vector.tensor_copy(dhi, tp[:])
                        nc.vector.tensor_tensor(dlo, tp[:], dhi, op=ALU.subtract)

                # pre-touch ACT function tables so mid-kernel LoadActFuncSet
                # swaps don't land on the critical path
                actw = ipp.tile([1, 4], F32, tag="ip", name="actw")
                for fn in (AF.Relu, AF.Exp, AF.Square, AF.Sqrt):
                    nc.scalar.activation(actw[0:1, 0:1], par[0:1, 0:1], fn)

                pe_backlog = []
                for b in range(B_LOC):
                    xts = [
                        xp.tile([128, H * W], BF16, tag="x", name=f"x_{b}_{i}")
                        for i in range(4)
                    ]
                    for cc in range(4):
                        if b == 0:
                            load_w_chunk(cc)
                        nc.sync.dma_start(
                            out=xts[cc][:],
                            in_=x_d.ap()[b, cc * 128:(cc + 1) * 128, :, :].rearrange(
                                "p h w -> p (h w)"
                            ),
                        )
                        if b == 0 and cc == 0:
                            load_par()
                    if b == 1:
                        load_fc_weights()
                    if b == 2:
                        load_lw_prefix()
                    agg = pa1.tile([KG, MID], F32, tag="agg", name=f"agg{b}")
                    ssum = pa1.tile([KG, 1], F32, tag="ssum", name=f"ssum{b}")
                    for wt in range(WT):
                        fps = pa2.tile([WSZ, 512], F32, tag="fps", name=f"fps{b}_{wt}")
                        sps = pa2.tile([WSZ, KG], F32, tag="sps", name=f"sps{b}_{wt}")
                        conv_mms(wt, fps, sps, xts)
                        for th in pe_backlog:
                            th()
                        pe_backlog = []
                        f_sb, sa_sb = postproc(fps, sps)
                        if wt < 3:
                            pe_backlog.append(
                                lambda wt=wt, f=f_sb, s=sa_sb, a=agg, ss=ssum: vlad_mms(
                                    wt, f, s, a, ss
                                )
                            )
                        else:

                            def end_of_image(
                                b=b, f=f_sb, s=sa_sb, a=agg, ss=ssum
                            ):
                                vlad_mms(3, f, s, a, ss)
                                image_post(b, a, ss)
                                if b == B_LOC - 1:
                                    # idle-fill while the last image's L2 DVE
                                    # chain runs; reuses a freed fps psum slot
                                    wrm = pa2.tile(
                                        [WSZ, 512], F32, tag="fps", name="wrm"
                                    )
                                    for wi in range(10):
                                        nc.tensor.matmul(
                                            wrm[0:4, :], ones_sb[:, 0:4],
                                            fcw_sb[:, 512:1024],
                                            start=(wi == 0), stop=(wi == 9),
                                            skip_group_check=True,
                                        )
                                transposes(b)

                            pe_backlog.append(end_of_image)
                for th in pe_backlog:
                    th()

            # ---- tail: fc -> AllGather -> BN+relu -> logit ----
            with (
                tc.tile_pool(name="tl", bufs=1) as tl,
                tc.tile_pool(name="bn2", bufs=2) as bn2,
                tc.tile_pool(name="pb2", bufs=2, space="PSUM") as pb2,
                tc.tile_pool(name="pb1", bufs=1, space="PSUM") as pb1,
            ):
                lw_tiles = {}
                for n in range(7, 12):
                    t = tl.tile([128, 2048], BF16, tag=f"lwt{n}", name=f"lwt{n}")
                    nc.sync.dma_start(
                        out=t[:], in_=lw_d.ap()[:, n * 2048:(n + 1) * 2048]
                    )
                    lw_tiles[n] = t

                h_ps = pb1.tile([B_LOC, MID], F32, tag="h")
                t_order = [k * 4 + cc for cc in range(4) for k in range(8)]
                for ti, t in enumerate(t_order):
                    for part, e0 in ((0, emb0T_hi), (1, emb0T_lo)):
                        nc.tensor.matmul(
                            h_ps[:],
                            e0[:, t * 4:(t + 1) * 4],
                            fcw_sb[:, t * 512:(t + 1) * 512],
                            start=(ti == 0 and part == 0),
                            stop=(ti == 31 and part == 1),
                            skip_group_check=True,
                        )
                h_sb = tl.tile([B_LOC, MID], F32, tag="h_sb")
                nc.vector.tensor_copy(h_sb[:], h_ps[:])
                cc_in = dp.tile([B_LOC, MID], F32)
                cc_out = dp.tile([32, MID], F32)
                if sim:
                    # timing stand-in for the AllGather (single-core sim)
                    nc.gpsimd.dma_start(cc_out[0:B_LOC, :], h_sb[:])
                else:
                    nc.sync.dma_start(out=cc_in[:], in_=h_sb[:])
                    nc.gpsimd.collective_compute(
                        "AllGather",
                        ALU.bypass,
                        replica_groups=[list(range(N_CORES))],
                        ins=[cc_in.opt()],
                        outs=[cc_out.opt()],
                    )
                # keep the PE HAM-warm across the AllGather latency window
                warm_ps = pb1.tile([4, 512], F32, tag="warm")
                for wi in range(20):
                    nc.tensor.matmul(
                        warm_ps[:], ones_sb[:, 0:4], fcw_sb[:, 0:512],
                        start=(wi == 0), stop=(wi == 19), skip_group_check=True,
                    )
                hall = tl.tile([32, MID], F32, tag="hall")
                nc.sync.dma_start(out=hall[:], in_=cc_out[:])

                for oc in range(4):
                    htp = pb2.tile([128, 32], F32, tag="htp", name=f"htp{oc}", bufs=2)
                    nc.tensor.transpose(
                        htp[:], hall[:, oc * 128:(oc + 1) * 128], idf_sb[0:32, 0:32]
                    )
                    # bn pack (f32): sm 0, s2 1, mean 2, ms 3, v1 4, var 5,
                    # vare 6, std 7, istd 8, scl 9, msc 10, sh 11,
                    # sqs [12:44), ef [44:76)
                    bp = bn2.tile([128, 76], F32, tag="bp", name=f"bp{oc}")
                    sm = bp[:, 0:1]
                    nc.vector.reduce_sum(
                        out=sm, in_=htp[:], axis=mybir.AxisListType.X
                    )
                    sqs = bp[:, 12:44]
                    s2 = bp[:, 1:2]
                    nc.scalar.activation(sqs, htp[:], AF.Square, accum_out=s2)
                    mean = bp[:, 2:3]
                    nc.vector.tensor_scalar_mul(mean, sm, 1.0 / 32.0)
                    ms = bp[:, 3:4]
                    nc.scalar.activation(ms, mean, AF.Square)
                    v1 = bp[:, 4:5]
                    nc.vector.tensor_scalar_mul(v1, s2, 1.0 / 32.0)
                    var = bp[:, 5:6]
                    nc.vector.tensor_tensor(var, v1, ms, op=ALU.subtract)
                    vare = bp[:, 6:7]
                    nc.vector.tensor_scalar_add(vare, var, EPS_BN)
                    std = bp[:, 7:8]
                    nc.scalar.activation(std, vare, AF.Sqrt)
                    istd = bp[:, 8:9]
                    nc.vector.reciprocal(istd, std)
                    scl = bp[:, 9:10]
                    nc.vector.tensor_tensor(
                        scl, istd, gam_sb[:, oc:oc + 1], op=ALU.mult
                    )
                    msc = bp[:, 10:11]
                    nc.vector.tensor_tensor(msc, mean, scl, op=ALU.mult)
                    sh = bp[:, 11:12]
                    nc.vector.tensor_tensor(
                        sh, bet_sb[:, oc:oc + 1], msc, op=ALU.subtract
                    )
                    ef = bp[:, 44:76]
                    nc.scalar.activation(ef, htp[:], AF.Relu, bias=sh, scale=scl)
                    nc.sync.dma_start(
                        out=emb_d.ap()[oc * 128:(oc + 1) * 128, :], in_=ef
                    )
                    nc.vector.tensor_copy(embT_bf[:, oc * 32:(oc + 1) * 32], ef)

                for n in range(12):
                    n0 = n * 512
                    ncols = min(512, OUT - n0)
                    lsrc = (
                        lwpre[:, n * 2048:(n + 1) * 2048]
                        if n < 7 else lw_tiles[n][:]
                    )
                    v_ps = pb2.tile([32, 512], F32, tag="v", name=f"v{n}")
                    for kt in range(4):
                        nc.tensor.matmul(
                            v_ps[:, 0:ncols],
                            embT_bf[:, kt * 32:(kt + 1) * 32],
                            lsrc[:, kt * 512: kt * 512 + ncols],
                            start=(kt == 0), stop=(kt == 3), skip_group_check=True,
                        )
                    v_sb = bn2.tile([32, 512], F32, tag="v_sb", name=f"vsb{n}")
                    nc.vector.tensor_copy(v_sb[:, 0:ncols], v_ps[:, 0:ncols])
                    nc.sync.dma_start(
                        out=vlad_d.ap()[:, n0:n0 + ncols], in_=v_sb[:, 0:ncols]
                    )

    nc.compile()
    return nc


def _prep_host(inputs):
    conv_w = np.asarray(inputs["conv_w"], dtype=np.float32)
    cc_w = np.asarray(inputs["cc_w"], dtype=np.float32)
    conv_b = np.asarray(inputs["conv_b"], dtype=np.float32)
    cc_b = np.asarray(inputs["cc_b"], dtype=np.float32)
    fc_w = np.asarray(inputs["fc_w"], dtype=np.float32)
    logit_w = np.asarray(inputs["logit_w"], dtype=np.float32)

    # combined conv weights -> [128, kt=28, m=522], kt = cc*7 + kh
    Wc = np.concatenate([conv_w[:, :, :, 0], cc_w[:, :, :, 0]], axis=0)  # [522,512,7]
    wt2 = (
        Wc.transpose(1, 2, 0)                  # [512, 7, 522] = (ci, kh, m)
        .reshape(4, 128, 7, 522)
        .transpose(1, 0, 2, 3)                 # [128, 4, 7, 522]
        .reshape(128, NKT * 522)
        .astype(BF16_NP)
    )
    fcw2 = (
        fc_w.reshape(32, 128, 512).transpose(1, 0, 2).reshape(128, 32 * 512)
    ).astype(BF16_NP)
    lw3 = np.zeros((128, 12 * 2048), dtype=BF16_NP)
    Lk = logit_w.reshape(4, 128, OUT).astype(BF16_NP)   # [kt, p, o]
    for n in range(12):
        n0 = n * 512
        ncols = min(512, OUT - n0)
        for kt in range(4):
            lw3[:, n * 2048 + kt * 512: n * 2048 + kt * 512 + ncols] = (
                Lk[kt, :, n0:n0 + ncols]
            )

    par = np.zeros((128, 973), dtype=np.float32)
    par[:, 0:128] = np.eye(128, dtype=np.float32)
    par[:, 128:132] = np.asarray(inputs["bn_gamma"], np.float32).reshape(4, 128).T
    par[:, 132:136] = np.asarray(inputs["bn_beta"], np.float32).reshape(4, 128).T
    par_u16 = par.view(np.uint16)
    par_u16[:, 272:400] = np.ones((128, 128), dtype=BF16_NP).view(np.uint16)
    par_u16[0, 400:922] = (
        np.concatenate([conv_b, cc_b]).astype(BF16_NP).view(np.uint16)
    )
    par[0:KG, 461:973] = np.asarray(inputs["centroids"], dtype=np.float32)

    common = {
        "wt2": wt2, "fcw2": fcw2, "lw3": lw3, "par": par,
    }
    feats = np.asarray(inputs["features"], dtype=np.float32).astype(BF16_NP)
    in_maps = []
    for c in range(N_CORES):
        m = dict(common)
        m["x"] = np.ascontiguousarray(feats[c * B_LOC:(c + 1) * B_LOC])
        in_maps.append(m)
    return in_maps


LAST_RESULTS = None


def kernel(**inputs):
    global LAST_RESULTS
    with_bias = bool(
        np.any(np.asarray(inputs["conv_b"])) or np.any(np.asarray(inputs["cc_b"]))
    )
    key = ("nc", with_bias)
    if key not in _CACHE:
        _CACHE[key] = _build(with_bias=with_bias)
    nc = _CACHE[key]
    in_maps = _prep_host(inputs)
    trace = os.environ.get("KBENCH_TRACE") == "1"
    try:
        res = run_bass_kernel_spmd(nc, in_maps, list(range(N_CORES)), trace=trace)
    except Exception:
        # transient NRT_EXEC_UNIT_UNRECOVERABLE flakes have been observed on
        # this fabric; one retry on a fresh execute clears them
        res = run_bass_kernel_spmd(nc, in_maps, list(range(N_CORES)), trace=trace)
    LAST_RESULTS = res
    r0 = res.results[0]
    embeddings = np.ascontiguousarray(r0["embT"].T)      # [32, 512]
    vlad = np.ascontiguousarray(r0["vlad"])              # [32, 5994]
    return embeddings, vlad


# revision 24
# speedup vs baseline: 1.0044x; 1.0044x over previous
"""Trainium2 Bass kernel for the gvlad pooling decoder (nn_Decoder_38182259261791).

Sharding: data-parallel over batch B=32 across 8 NeuronCores (4 images/core).
Per core: conv(7x1)+relu and cluster-score conv are fused into one matmul over
the contracted (C_in*kh)=3584 axis, emitted transposed ([w, c] layout) so the
VLAD aggregation needs no on-chip transpose. BatchNorm batch stats come from an
8-core AllGather of the per-core fc outputs h[4,512]; every core then computes
the full tail (BN + relu + logit) redundantly and core 0's output is returned.
"""

import os

import numpy as np
import ml_dtypes

import concourse.bacc as bacc
import concourse.mybir as mybir
from concourse.tile import TileContext
from concourse.bass_utils import run_bass_kernel_spmd

N_CORES = 8
B_LOC = 4            # images per core
C = 512
H = 7
W = 500
KG = 10              # clusters incl. ghost
K_CL = 8             # kept clusters
NKT = 28             # contraction K-tiles: 4 ci-chunks x 7 kh
MID = 512
OUT = 5994
WT = 4               # w-tiles per image
WSZ = 125            # w-tile size
EPS_BN = 1e-5

BF16 = mybir.dt.bfloat16
F32 = mybir.dt.float32
AF = mybir.ActivationFunctionType
ALU = mybir.AluOpType
BF16_NP = ml_dtypes.bfloat16

_CACHE = {}


def _build(sim=False, with_bias=True):
    nc = bacc.Bacc(
        "TRN2", target_bir_lowering=False, debug=False,
        num_devices=1 if sim else N_CORES,
    )

    x_d = nc.dram_tensor("x", [B_LOC, C, H, W], BF16, kind="ExternalInput")
    w_d = nc.dram_tensor("wt2", [128, NKT * 522], BF16, kind="ExternalInput")
    fcw_d = nc.dram_tensor("fcw2", [128, 32 * 512], BF16, kind="ExternalInput")
    # n-major logit weights: chunk n at [n*2048, (n+1)*2048), kt-major inside
    lw_d = nc.dram_tensor("lw3", [128, 12 * 2048], BF16, kind="ExternalInput")
    # packed f32 params: [0:128) identity, [128:132) gamma, [132:136) beta,
    # [136:200) ones (bf16 x128 via bitcast), [200:461) conv bias row (bf16
    # x522 via bitcast on partition 0), [461:973) centroids (rows 0:10)
    par_d = nc.dram_tensor("par", [128, 973], F32, kind="ExternalInput")

    emb_d = nc.dram_tensor("embT", [MID, 32], F32, kind="ExternalOutput")
    vlad_d = nc.dram_tensor("vlad", [32, OUT], F32, kind="ExternalOutput")

    with TileContext(nc) as tc:
        with (
            tc.tile_pool(name="wp", bufs=1) as wp,
            tc.tile_pool(name="dram", bufs=1, space="DRAM") as dp,
        ):
            # ---- persistent SBUF ----
            w_sb = wp.tile([128, NKT * 522], BF16, tag="w")
            csz = 7 * 522

            def load_w_chunk(i):
                nc.sync.dma_start(
                    out=w_sb[:, i * csz:(i + 1) * csz],
                    in_=w_d.ap()[:, i * csz:(i + 1) * csz],
                )
            par = wp.tile([128, 973], F32, tag="par")

            def load_par():
                nc.sync.dma_start(out=par[:], in_=par_d.ap())

            idf_sb = par[:, 0:128]
            gam_sb = par[:, 128:132]
            bet_sb = par[:, 132:136]
            ones_sb = par[:, 136:200].bitcast(BF16)
            cb_sb = par[0:1, 200:461].bitcast(BF16)
            cent_sb = par[0:KG, 461:973]
            fcw_sb = wp.tile([128, 32 * 512], BF16, tag="fcw")
            lwpre = wp.tile([128, 7 * 2048], BF16, tag="lwpre")
            # embpack: [0:128) emb0T hi, [128:256) emb0T lo, [256:384) embT
            embpack = wp.tile([128, 384], BF16, tag="embpack")
            emb0T_hi = embpack[:, 0:128]
            emb0T_lo = embpack[:, 128:256]
            embT_bf = embpack[:, 256:384]
            cl2all = wp.tile([K_CL, B_LOC * MID], F32, tag="cl2all")

            def load_fc_weights():
                fsz = 8 * 512
                for i in range(4):
                    nc.sync.dma_start(
                        out=fcw_sb[:, i * fsz:(i + 1) * fsz],
                        in_=fcw_d.ap()[:, i * fsz:(i + 1) * fsz],
                    )

            def load_lw_prefix():
                for i in range(2):
                    nc.sync.dma_start(
                        out=lwpre[:, i * 7168:(i + 1) * 7168],
                        in_=lw_d.ap()[:, i * 7168:(i + 1) * 7168],
                    )

            # ---- conv + VLAD phase ----
            with (
                tc.tile_pool(name="xp", bufs=8) as xp,
                tc.tile_pool(name="scr", bufs=3) as scr,
                tc.tile_pool(name="ip", bufs=1) as ipp,
                tc.tile_pool(name="pa2", bufs=2, space="PSUM") as pa2,
                tc.tile_pool(name="pa1", bufs=1, space="PSUM") as pa1,
            ):

                def conv_mms(wt, fps, sps, xts):
                    for kt in range(NKT):
                        cc, kh = divmod(kt, 7)
                        lhs = xts[cc][:, kh * W + wt * WSZ: kh * W + (wt + 1) * WSZ]
                        nc.tensor.matmul(
                            fps[:], lhs, w_sb[:, kt * 522: kt * 522 + 512],
                            start=(kt == 0),
                            stop=(not with_bias and kt == NKT - 1),
                            skip_group_check=True,
                        )
                        nc.tensor.matmul(
                            sps[:], lhs, w_sb[:, kt * 522 + 512: (kt + 1) * 522],
                            start=(kt == 0),
                            stop=(not with_bias and kt == NKT - 1),
                            skip_group_check=True,
                        )
                    if with_bias:
                        nc.tensor.matmul(
                            fps[:], ones_sb[0:1, 0:WSZ], cb_sb[0:1, 0:512],
                            start=False, stop=True, skip_group_check=True,
                        )
                        nc.tensor.matmul(
                            sps[:], ones_sb[0:1, 0:WSZ], cb_sb[0:1, 512:522],
                            start=False, stop=True, skip_group_check=True,
                        )

                def postproc(fps, sps):
                    # packed scratch: f bf16 [0:256)f32, e [256:266), rs 266,
                    # inv 267, sa bf16 [268:273)
                    pk = scr.tile([WSZ, 276], F32, tag="pk")
                    f_sb = pk[:, 0:256].bitcast(BF16)
                    nc.scalar.activation(f_sb, fps[:], AF.Relu)
                    e_sb = pk[:, 256:266]
                    rs = pk[:, 266:267]
                    nc.scalar.activation(e_sb, sps[:], AF.Exp, accum_out=rs)
                    inv = pk[:, 267:268]
                    nc.vector.reciprocal(inv, rs)
                    sa_sb = pk[:, 268:273].bitcast(BF16)
                    nc.vector.tensor_scalar_mul(sa_sb, e_sb, inv)
                    return f_sb, sa_sb

                def vlad_mms(wt, f_sb, sa_sb, agg, ssum):
                    nc.tensor.matmul(
                        agg[:], sa_sb, f_sb,
                        start=(wt == 0), stop=(wt == 3), skip_group_check=True,
                    )
                    nc.tensor.matmul(
                        ssum[:], sa_sb, ones_sb[0:WSZ, 0:1],
                        start=(wt == 0), stop=(wt == 3), skip_group_check=True,
                    )

                def image_post(b, agg, ssum):
                    # ip pack (f32): tmp [0:512), res [512:1024), sq [1024:1536),
                    # ss 1536, qs 1537, nrm 1538, nrm2 1539, invn 1540
                    ip = ipp.tile([KG, 1544], F32, tag="ip")
                    ss_sb = ip[:, 1536:1537]
                    nc.vector.tensor_copy(ss_sb, ssum[:])
                    tmp = ip[:, 0:512]
                    nc.vector.tensor_scalar_mul(tmp, cent_sb[:], ss_sb)
                    res = ip[:, 512:1024]
                    nc.vector.tensor_tensor(res, agg[:], tmp, op=ALU.subtract)
                    sq = ip[0:K_CL, 1024:1536]
                    qs = ip[0:K_CL, 1537:1538]
                    nc.scalar.activation(
                        sq, res[0:K_CL, :], AF.Square, accum_out=qs
                    )
                    nrm = ip[0:K_CL, 1538:1539]
                    nc.scalar.activation(nrm, qs, AF.Sqrt)
                    nrm2 = ip[0:K_CL, 1539:1540]
                    nc.vector.tensor_scalar_max(nrm2, nrm, 1e-12)
                    invn = ip[0:K_CL, 1540:1541]
                    nc.vector.reciprocal(invn, nrm2)
                    nc.vector.tensor_scalar_mul(
                        cl2all[:, b * MID:(b + 1) * MID], res[0:K_CL, :], invn
                    )

                def transposes(b):
                    # cl2[b] [8, 512] f32 -> emb0T hi/lo bf16 cols k*16 + cc*4 + b
                    for cc4 in range(4):
                        tp = pa2.tile([128, K_CL], F32, tag="tp", name=f"tp{b}_{cc4}")
                        nc.tensor.transpose(
                            tp[:],
                            cl2all[:, b * MID + cc4 * 128: b * MID + (cc4 + 1) * 128],
                            idf_sb[0:K_CL, 0:K_CL],
                        )
                        dhi = emb0T_hi.rearrange("p (k s) -> p k s", s=16)[
                            :, :, cc4 * 4 + b
                        ]
                        dlo = emb0T_lo.rearrange("p (k s) -> p k s", s=16)[
                            :, :, cc4 * 4 + b
                        ]
                        nc.vector.tensor_copy(dhi, tp[:])
                        nc.vector.tensor_tensor(dlo, tp[:], dhi, op=ALU.subtract)

                # pre-touch ACT function tables so mid-kernel LoadActFuncSet
                # swaps don't land on the critical path
                actw = ipp.tile([1, 4], F32, tag="ip", name="actw")
                for fn in (AF.Relu, AF.Exp, AF.Square, AF.Sqrt):
                    nc.scalar.activation(actw[0:1, 0:1], par[0:1, 0:1], fn)

                pe_backlog = []
                for b in range(B_LOC):
                    xts = [
                        xp.tile([128, H * W], BF16, tag="x", name=f"x_{b}_{i}")
                        for i in range(4)
                    ]
                    for cc in range(4):
                        if b == 0:
                            load_w_chunk(cc)
                        nc.sync.dma_start(
                            out=xts[cc][:],
                            in_=x_d.ap()[b, cc * 128:(cc + 1) * 128, :, :].rearrange(
                                "p h w -> p (h w)"
                            ),
                        )
                        if b == 0 and cc == 0:
                            load_par()
                    if b == 1:
                        load_fc_weights()
                    if b == 2:
                        load_lw_prefix()
                    agg = pa1.tile([KG, MID], F32, tag="agg", name=f"agg{b}")
                    ssum = pa1.tile([KG, 1], F32, tag="ssum", name=f"ssum{b}")
                    for wt in range(WT):
                        fps = pa2.tile([WSZ, 512], F32, tag="fps", name=f"fps{b}_{wt}")
                        sps = pa2.tile([WSZ, KG], F32, tag="sps", name=f"sps{b}_{wt}")
                        conv_mms(wt, fps, sps, xts)
                        for th in pe_backlog:
                            th()
                        pe_backlog = []
                        f_sb, sa_sb = postproc(fps, sps)
                        if wt < 3:
                            pe_backlog.append(
                                lambda wt=wt, f=f_sb, s=sa_sb, a=agg, ss=ssum: vlad_mms(
                                    wt, f, s, a, ss
                                )
                            )
                        else:

                            def end_of_image(
                                b=b, f=f_sb, s=sa_sb, a=agg, ss=ssum
                            ):
                                vlad_mms(3, f, s, a, ss)
                                image_post(b, a, ss)
                                if b == B_LOC - 1:
                                    # idle-fill while the last image's L2 DVE
                                    # chain runs; reuses a freed fps psum slot
                                    wrm = pa2.tile(
                                        [WSZ, 512], F32, tag="fps", name="wrm"
                                    )
                                    for wi in range(10):
                                        nc.tensor.matmul(
                                            wrm[0:4, :], ones_sb[:, 0:4],
                                            fcw_sb[:, 512:1024],
                                            start=(wi == 0), stop=(wi == 9),
                                            skip_group_check=True,
                                        )
                                transposes(b)

                            pe_backlog.append(end_of_image)
                for th in pe_backlog:
                    th()

            # ---- tail: fc -> AllGather -> BN+relu -> logit ----
            with (
                tc.tile_pool(name="tl", bufs=1) as tl,
                tc.tile_pool(name="bn2", bufs=2) as bn2,
                tc.tile_pool(name="pb2", bufs=2, space="PSUM") as pb2,
                tc.tile_pool(name="pb1", bufs=1, space="PSUM") as pb1,
            ):
                lw_tiles = {}
                for n in range(7, 12):
                    t = tl.tile([128, 2048], BF16, tag=f"lwt{n}", name=f"lwt{n}")
                    nc.sync.dma_start(
                        out=t[:], in_=lw_d.ap()[:, n * 2048:(n + 1) * 2048]
                    )
                    lw_tiles[n] = t

                h_ps = pb1.tile([B_LOC, MID], F32, tag="h")
                t_order = [k * 4 + cc for cc in range(4) for k in range(8)]
                for ti, t in enumerate(t_order):
                    for part, e0 in ((0, emb0T_hi), (1, emb0T_lo)):
                        nc.tensor.matmul(
                            h_ps[:],
                            e0[:, t * 4:(t + 1) * 4],
                            fcw_sb[:, t * 512:(t + 1) * 512],
                            start=(ti == 0 and part == 0),
                            stop=(ti == 31 and part == 1),
                            skip_group_check=True,
                        )
                h_sb = tl.tile([B_LOC, MID], F32, tag="h_sb")
                nc.vector.tensor_copy(h_sb[:], h_ps[:])
                cc_in = dp.tile([B_LOC, MID], F32)
                cc_out = dp.tile([32, MID], F32)
                if sim:
                    # timing stand-in for the AllGather (single-core sim)
                    nc.gpsimd.dma_start(cc_out[0:B_LOC, :], h_sb[:])
                else:
                    nc.sync.dma_start(out=cc_in[:], in_=h_sb[:])
                    nc.gpsimd.collective_compute(
                        "AllGather",
                        ALU.bypass,
                        replica_groups=[list(range(N_CORES))],
                        ins=[cc_in.opt()],
                        outs=[cc_out.opt()],
                    )
                # keep the PE HAM-warm across the AllGather latency window
                warm_ps = pb1.tile([4, 512], F32, tag="warm")
                for wi in range(20):
                    nc.tensor.matmul(
                        warm_ps[:], ones_sb[:, 0:4], fcw_sb[:, 0:512],
                        start=(wi == 0), stop=(wi == 19), skip_group_check=True,
                    )
                hall = tl.tile([32, MID], F32, tag="hall")
                htps = []
                for oc in range(4):
                    nc.sync.dma_start(
                        out=hall[:, oc * 128:(oc + 1) * 128],
                        in_=cc_out[:, oc * 128:(oc + 1) * 128],
                    )
                    htp = pb2.tile([128, 32], F32, tag="htp", name=f"htp{oc}", bufs=4)
                    nc.tensor.transpose(
                        htp[:], hall[:, oc * 128:(oc + 1) * 128], idf_sb[0:32, 0:32]
                    )
                    htps.append(htp)

                for oc in range(4):
                    htp = htps[oc]
                    # bn pack (f32): sm 0, s2 1, mean 2, ms 3, v1 4, var 5,
                    # vare 6, std 7, istd 8, scl 9, msc 10, sh 11,
                    # sqs [12:44), ef [44:76)
                    bp = bn2.tile([128, 76], F32, tag="bp", name=f"bp{oc}")
                    sm = bp[:, 0:1]
                    nc.vector.reduce_sum(
                        out=sm, in_=htp[:], axis=mybir.AxisListType.X
                    )
                    sqs = bp[:, 12:44]
                    s2 = bp[:, 1:2]
                    nc.scalar.activation(sqs, htp[:], AF.Square, accum_out=s2)
                    mean = bp[:, 2:3]
                    nc.vector.tensor_scalar_mul(mean, sm, 1.0 / 32.0)
                    ms = bp[:, 3:4]
                    nc.scalar.activation(ms, mean, AF.Square)
                    v1 = bp[:, 4:5]
                    nc.vector.tensor_scalar_mul(v1, s2, 1.0 / 32.0)
                    var = bp[:, 5:6]
                    nc.vector.tensor_tensor(var, v1, ms, op=ALU.subtract)
                    vare = bp[:, 6:7]
                    nc.vector.tensor_scalar_add(vare, var, EPS_BN)
                    std = bp[:, 7:8]
                    nc.scalar.activation(std, vare, AF.Sqrt)
                    istd = bp[:, 8:9]
                    nc.vector.reciprocal(istd, std)
                    scl = bp[:, 9:10]
                    nc.vector.tensor_tensor(
                        scl, istd, gam_sb[:, oc:oc + 1], op=ALU.mult
                    )
                    msc = bp[:, 10:11]
                    nc.vector.tensor_tensor(msc, mean, scl, op=ALU.mult)
                    sh = bp[:, 11:12]
                    nc.vector.tensor_tensor(
                        sh, bet_sb[:, oc:oc + 1], msc, op=ALU.subtract
                    )
                    ef = bp[:, 44:76]
                    nc.scalar.activation(ef, htp[:], AF.Relu, bias=sh, scale=scl)
                    nc.sync.dma_start(
                        out=emb_d.ap()[oc * 128:(oc + 1) * 128, :], in_=ef
                    )
                    nc.vector.tensor_copy(embT_bf[:, oc * 32:(oc + 1) * 32], ef)

                for n in range(12):
                    n0 = n * 512
                    ncols = min(512, OUT - n0)
                    lsrc = (
                        lwpre[:, n * 2048:(n + 1) * 2048]
                        if n < 7 else lw_tiles[n][:]
                    )
                    v_ps = pb2.tile([32, 512], F32, tag="v", name=f"v{n}")
                    for kt in range(4):
                        nc.tensor.matmul(
                            v_ps[:, 0:ncols],
                            embT_bf[:, kt * 32:(kt + 1) * 32],
                            lsrc[:, kt * 512: kt * 512 + ncols],
                            start=(kt == 0), stop=(kt == 3), skip_group_check=True,
                        )
                    v_sb = bn2.tile([32, 512], F32, tag="v_sb", name=f"vsb{n}")
                    nc.vector.tensor_copy(v_sb[:, 0:ncols], v_ps[:, 0:ncols])
                    nc.sync.dma_start(
                        out=vlad_d.ap()[:, n0:n0 + ncols], in_=v_sb[:, 0:ncols]
                    )

    nc.compile()
    return nc


def _prep_host(inputs):
    conv_w = np.asarray(inputs["conv_w"], dtype=np.float32)
    cc_w = np.asarray(inputs["cc_w"], dtype=np.float32)
    conv_b = np.asarray(inputs["conv_b"], dtype=np.float32)
    cc_b = np.asarray(inputs["cc_b"], dtype=np.float32)
    fc_w = np.asarray(inputs["fc_w"], dtype=np.float32)
    logit_w = np.asarray(inputs["logit_w"], dtype=np.float32)

    # combined conv weights -> [128, kt=28, m=522], kt = cc*7 + kh
    Wc = np.concatenate([conv_w[:, :, :, 0], cc_w[:, :, :, 0]], axis=0)  # [522,512,7]
    wt2 = (
        Wc.transpose(1, 2, 0)                  # [512, 7, 522] = (ci, kh, m)
        .reshape(4, 128, 7, 522)
        .transpose(1, 0, 2, 3)                 # [128, 4, 7, 522]
        .reshape(128, NKT * 522)
        .astype(BF16_NP)
    )
    fcw2 = (
        fc_w.reshape(32, 128, 512).transpose(1, 0, 2).reshape(128, 32 * 512)
    ).astype(BF16_NP)
    lw3 = np.zeros((128, 12 * 2048), dtype=BF16_NP)
    Lk = logit_w.reshape(4, 128, OUT).astype(BF16_NP)   # [kt, p, o]
    for n in range(12):
        n0 = n * 512
        ncols = min(512, OUT - n0)
        for kt in range(4):
            lw3[:, n * 2048 + kt * 512: n * 2048 + kt * 512 + ncols] = (
                Lk[kt, :, n0:n0 + ncols]
            )

    par = np.zeros((128, 973), dtype=np.float32)
    par[:, 0:128] = np.eye(128, dtype=np.float32)
    par[:, 128:132] = np.asarray(inputs["bn_gamma"], np.float32).reshape(4, 128).T
    par[:, 132:136] = np.asarray(inputs["bn_beta"], np.float32).reshape(4, 128).T
    par_u16 = par.view(np.uint16)
    par_u16[:, 272:400] = np.ones((128, 128), dtype=BF16_NP).view(np.uint16)
    par_u16[0, 400:922] = (
        np.concatenate([conv_b, cc_b]).astype(BF16_NP).view(np.uint16)
    )
    par[0:KG, 461:973] = np.asarray(inputs["centroids"], dtype=np.float32)

    common = {
        "wt2": wt2, "fcw2": fcw2, "lw3": lw3, "par": par,
    }
    feats = np.asarray(inputs["features"], dtype=np.float32).astype(BF16_NP)
    in_maps = []
    for c in range(N_CORES):
        m = dict(common)
        m["x"] = np.ascontiguousarray(feats[c * B_LOC:(c + 1) * B_LOC])
        in_maps.append(m)
    return in_maps


LAST_RESULTS = None


def kernel(**inputs):
    global LAST_RESULTS
    with_bias = bool(
        np.any(np.asarray(inputs["conv_b"])) or np.any(np.asarray(inputs["cc_b"]))
    )
    key = ("nc", with_bias)
    if key not in _CACHE:
        _CACHE[key] = _build(with_bias=with_bias)
    nc = _CACHE[key]
    in_maps = _prep_host(inputs)
    trace = os.environ.get("KBENCH_TRACE") == "1"
    try:
        res = run_bass_kernel_spmd(nc, in_maps, list(range(N_CORES)), trace=trace)
    except Exception:
        # transient NRT_EXEC_UNIT_UNRECOVERABLE flakes have been observed on
        # this fabric; one retry on a fresh execute clears them
        res = run_bass_kernel_spmd(nc, in_maps, list(range(N_CORES)), trace=trace)
    LAST_RESULTS = res
    r0 = res.results[0]
    embeddings = np.ascontiguousarray(r0["embT"].T)      # [32, 512]
    vlad = np.ascontiguousarray(r0["vlad"])              # [32, 5994]
    return embeddings, vlad
